# revision 6
# baseline (speedup 1.0000x reference)
"""Trainium2 Bass kernel for nn_Decay2D (decay-masked linear attention).

Math: the reference's Hillis-Steele scan with decay-squaring order composes
to coefficient d^ceil((t-s)/2) on store[s] = scale*k_s v_s^T, so

    out[t] = scale^2 * sum_{s<=t} d^ceil((t-s)/2) (q_t . k_s) v_s  @ Wo^T

computed as chunked linear attention with two [K,V] carry states (even/odd
decay chains), never materializing the [B,T,K,V] memory.

Sharding: 8 cores = 4 batches x 2 sequence halves. Each core builds the
carry state over a truncated 128-row prefix and runs full attention +
output projection for its own 1024 rows.

v4 changes vs v3 (50.2us): the body is restructured around the DMA stream.
Inputs arrive in compute order (weights, prefix, per-half-group x slices)
on the sync HWDGE ring; the PE chases the stream so matmuls stay dense
(HAM warm) and each chunk's output row block is stored as soon as its two
PSUM halves are copied. Constants shrink to one ident block + mloc/ce/co;
the per-chunk state weights (wge/wgo) ride gamma columns and are applied
with per-partition tensor_scalar ops, and k is transposed once per chunk.
"""

from contextlib import ExitStack

import numpy as np

import concourse.bass as bass
import concourse.bacc as bacc
import concourse.mybir as mybir
import concourse.tile as tile
from concourse import bass_utils
from concourse.alu_op_type import AluOpType
from concourse.bass import ts

F32 = mybir.dt.float32
BF16 = mybir.dt.bfloat16
SIG = mybir.ActivationFunctionType.Sigmoid

B, T, E, K, V = 4, 2048, 1024, 64, 64
DECAY = 0.9
C = 128          # chunk length
HT = T // 2      # rows per core (sequence half)
NCH = HT // C    # chunks per half (8)
NEC = E // 128   # embed sub-chunks (8)
GW = 512         # group width: 4 chunks per PSUM bank
GCH = GW // C    # chunks per group (4)
NG = HT // GW    # groups per half (2)
DC2 = float(DECAY ** (C // 2))
N_CORES = 8
PRE = 128        # truncated prefix length (1 chunk; older rows decay < 2e-3)

# hot-constants layout
def _mklayout(regions):
    out, off = {}, 0
    for n, r, c in regions:
        out[n] = (r, off, c)
        off += c
    return out, off


_IDB, IDB_W = _mklayout([("ident", 64, 64), ("identhi", 128, 64)])
_HOT, HOT_W = _mklayout([("mlocT4", C, GW), ("cemat", K, GW), ("comat", K, GW)])

# gamma columns: 0 prefix flag, 1 bk|bv, 2 bq, 3 wge, 4 wgo
G_GAMMA, G_BKV, G_BQ, G_WGE, G_WGO = 0, 1, 2, 3, 4
G_NCOL = 5


def _host_constants():
    d = DECAY
    scale2 = 1.0 - d
    i = np.arange(C)
    j = np.arange(C)
    delta = i[:, None] - j[None, :]
    # intra-chunk decay mask, transposed to [tcol(j), trow(i)], scale^2 folded
    mloc = np.where(delta >= 0, d ** np.ceil(delta / 2.0), 0.0) * scale2
    mlocT4 = np.tile(np.ascontiguousarray(mloc.T), (1, GCH)).astype(np.float32)
    # boundary coefficient per local row i (scale^2 folded), split by parity
    c = d ** np.ceil((i + 1) / 2.0) * scale2
    ce = np.where(i % 2 == 0, c, 0.0).astype(np.float32)
    co = np.where(i % 2 == 1, c, 0.0).astype(np.float32)
    cemat = np.tile(np.broadcast_to(ce, (K, C)), (1, GCH)).astype(np.float32)
    comat = np.tile(np.broadcast_to(co, (K, C)), (1, GCH)).astype(np.float32)
    # state-update row weights (per t within chunk)
    u_o = np.where(j % 2 == 1, d ** ((C - 1 - j) / 2.0), 0.0)
    u_e = np.where(j % 2 == 0, d ** ((C - 2 - j) / 2.0), 0.0)
    wge = (u_o + u_e).astype(np.float32)          # [C]
    wgo = (u_o + d * u_e).astype(np.float32)
    return {
        "mlocT4": mlocT4,
        "cemat": np.ascontiguousarray(cemat),
        "comat": np.ascontiguousarray(comat),
        "wge": wge,
        "wgo": wgo,
        "ident64": np.eye(64, dtype=np.float32),
    }


def _build_program(has_bv):
    nc = bacc.Bacc(
        "TRN2",
        debug=False,
        enable_asserts=False,
        target_bir_lowering=False,
        num_devices=N_CORES,
    )

    def din(name, shape, dtype=F32):
        return nc.dram_tensor(name, shape, dtype, kind="ExternalInput").ap()

    gamma_d = din("gamma_col", [128, G_NCOL])
    wpx_d = din("wpx", [128, NEC * 3 * K], BF16)      # packed Wk|Wv, Wq
    xpre_d = din("xpre", [128, NEC * PRE], BF16)      # prefix x, (ec, t)
    cid_d = din("cid", [128, IDB_W], BF16)            # ident blocks
    xq2 = din("xq2", [NG * 128, NEC * GW], BF16)      # x, (ec, half, t) per group
    chot_d = din("chot", [128, HOT_W], BF16)          # mloc / ce / co
    woT_d = din("woT", [V, E], BF16)
    out_d = nc.dram_tensor("out", [NCH, 128, E], BF16,
                           kind="ExternalOutput").ap()

    with ExitStack() as ctx:
        tc = ctx.enter_context(tc_ctx := tile.TileContext(nc))

        consts = ctx.enter_context(tc.tile_pool(name="consts", bufs=1))
        state = ctx.enter_context(tc.tile_pool(name="state", bufs=1))
        xpool = ctx.enter_context(tc.tile_pool(name="xg", bufs=2))
        spool = ctx.enter_context(tc.tile_pool(name="sml", bufs=2))
        opool = ctx.enter_context(tc.tile_pool(name="osb", bufs=3))
        # PSUM budget (8 banks): pproj ring 2, pbig ring 2, psml ring 2,
        # pstate 1 (pu2 lives across both groups)
        pproj = ctx.enter_context(tc.tile_pool(name="pproj", bufs=2, space="PSUM"))
        pbig = ctx.enter_context(tc.tile_pool(name="pbig", bufs=2, space="PSUM"))
        psml = ctx.enter_context(tc.tile_pool(name="psml", bufs=2, space="PSUM"))
        pstate = ctx.enter_context(tc.tile_pool(name="pstate", bufs=1, space="PSUM"))

        # ---- loads in wire order (single sync HWDGE ring = FIFO) ----
        gamma = consts.tile([128, G_NCOL], F32, name="gamma_sb")
        nc.sync.dma_start(gamma[:], gamma_d[:])
        wpx = consts.tile([128, NEC * 3 * K], BF16, name="wpx")
        nc.sync.dma_start(wpx[:], wpx_d[:])
        xp = consts.tile([128, NEC * PRE], BF16, name="xp")
        nc.sync.dma_start(xp[:], xpre_d[:])
        cid = consts.tile([128, IDB_W], BF16, name="cid")
        nc.sync.dma_start(cid[:], cid_d[:])

        xg2s = []
        for g in range(NG):
            xg = xpool.tile([128, NEC * GW], BF16, tag="xg", name=f"xg2_{g}")
            xg2s.append(xg)
        HCOL = (NEC // 2) * GW  # half the ec blocks

        def load_xhalf(g, hf):
            nc.sync.dma_start(
                xg2s[g][:, hf * HCOL : (hf + 1) * HCOL],
                xq2[g * 128 : (g + 1) * 128, hf * HCOL : (hf + 1) * HCOL],
            )

        load_xhalf(0, 0)
        load_xhalf(0, 1)
        chot = consts.tile([128, HOT_W], BF16, name="chot")
        nc.sync.dma_start(chot[:], chot_d[:])
        wo = consts.tile([V, E], BF16, name="wo")
        nc.sync.dma_start(wo[:], woT_d[:])
        load_xhalf(1, 0)
        load_xhalf(1, 1)

        def reg(pack, layout, name):
            r, o, c = layout[name]
            return pack[0:r, o : o + c]

        wkv = wpx[:, 0 : NEC * 2 * K]
        wq = wpx[:, NEC * 2 * K : NEC * 3 * K]
        ident, identhi = reg(cid, _IDB, "ident"), reg(cid, _IDB, "identhi")
        mlocT4 = reg(chot, _HOT, "mlocT4")
        cemat, comat = reg(chot, _HOT, "cemat"), reg(chot, _HOT, "comat")
        bk_ap = gamma[0:K, G_BKV : G_BKV + 1]
        bv_ap = gamma[K : 2 * K, G_BKV : G_BKV + 1]
        bq_ap = gamma[0:K, G_BQ : G_BQ + 1]
        wge_ap = gamma[:, G_WGE : G_WGE + 1]
        wgo_ap = gamma[:, G_WGO : G_WGO + 1]

        qT_all = consts.tile([K, HT], BF16, name="qT_all")
        kT_all = consts.tile([K, HT], BF16, name="kT_all")
        lt_all = consts.tile([V, HT], BF16, name="lt_all")
        geo_all = state.tile([2 * K, NCH * V], F32, name="geo_all")
        geo_bf = state.tile([2 * K, NCH * V], BF16, name="geo_bf")

        # ============ prefix projection (1 chunk) ============
        pkv1 = pproj.tile([2 * K, PRE], F32, tag="pP", name="pkv1")
        for ec in range(NEC):
            nc.tensor.matmul(pkv1[:], wkv[:, ts(ec, 2 * K)], xp[:, ts(ec, PRE)],
                             start=(ec == 0), stop=(ec == NEC - 1))
        kT1 = spool.tile([K, PRE], BF16, tag="kT1", name="kT1")
        nc.scalar.activation(kT1[:], pkv1[0:K, :], SIG, bias=bk_ap)
        vT1 = spool.tile([2 * K, PRE], BF16, tag="vT1", name="vT1")
        nc.scalar.copy(vT1[K : 2 * K, :], pkv1[K : 2 * K, :])
        if has_bv:
            nc.vector.tensor_scalar_add(vT1[K : 2 * K, :], vT1[K : 2 * K, :], bv_ap)

        # ============ group projections (PE chases the x stream) ============
        vT_sbs = []

        def proj_group(g):
            pkv = pproj.tile([2 * K, GW], F32, tag="pP", name=f"pkv_{g}")
            pqg = pproj.tile([K, GW], F32, tag="pP", name=f"pq_{g}")
            for hf in range(2):
                for e4 in range(NEC // 2):
                    ec = hf * (NEC // 2) + e4
                    nc.tensor.matmul(pkv[:], wkv[:, ts(ec, 2 * K)],
                                     xg2s[g][:, ts(ec, GW)],
                                     start=(ec == 0), stop=(ec == NEC - 1))
                for e4 in range(NEC // 2):
                    ec = hf * (NEC // 2) + e4
                    nc.tensor.matmul(pqg[:], wq[:, ts(ec, K)],
                                     xg2s[g][:, ts(ec, GW)],
                                     start=(ec == 0), stop=(ec == NEC - 1))
            nc.scalar.activation(kT_all[:, ts(g, GW)], pkv[0:K, :], SIG, bias=bk_ap)
            vT_sb = spool.tile([2 * K, GW], BF16, tag=f"vT{g}", name=f"vT_sb{g}")
            nc.scalar.copy(vT_sb[K : 2 * K, :], pkv[K : 2 * K, :])
            if has_bv:
                nc.vector.tensor_scalar_add(
                    vT_sb[K : 2 * K, :], vT_sb[K : 2 * K, :], bv_ap)
            vT_sbs.append(vT_sb)
            nc.scalar.activation(qT_all[:, ts(g, GW)], pqg[:], SIG, bias=bq_ap)

        # ---- stage helpers ----
        def transposes(kT_src, vT_src, nch, tagp):
            pkn = psml.tile([C, nch * K], BF16, tag="pS", name=f"pkn{tagp}")
            for cl in range(nch):
                nc.tensor.matmul(pkn[:, ts(cl, K)], kT_src[:, ts(cl, C)],
                                 ident[:], is_transpose=True)
            pvn = psml.tile([C, nch * V], BF16, tag="pS", name=f"pvn{tagp}")
            for cl in range(nch):
                nc.tensor.matmul(pvn[:, ts(cl, V)], vT_src[K : 2 * K, ts(cl, C)],
                                 identhi[K : 2 * K, :], is_transpose=True)
            kn = spool.tile([C, nch * K], BF16, tag=f"kn{tagp}", name=f"kn{tagp}")
            nc.scalar.copy(kn[:], pkn[:])
            v_b = spool.tile([C, nch * V], BF16, tag=f"v{tagp}", name=f"v{tagp}")
            nc.vector.tensor_copy(v_b[:], pvn[:])
            return kn, v_b

        def kgeo_stage(kn, nch, tagp):
            # kgeo[:, cl*2K : cl*2K+K] = kn_cl * wge ; [+K : +2K] = kn_cl * wgo
            kgeo = spool.tile([C, nch * 2 * K], BF16, tag=f"kg{tagp}",
                              name=f"kgeo{tagp}")
            kv3 = kgeo[:].rearrange("p (n two k) -> p n two k", two=2, k=K)
            ki3 = kn[:].rearrange("p (n k) -> p n k", k=K)
            nc.vector.tensor_scalar_mul(kv3[:, :, 0, :], ki3, wge_ap)
            nc.vector.tensor_scalar_mul(kv3[:, :, 1, :], ki3, wgo_ap)
            return kgeo

        def scores_stage(g):
            ps = pbig.tile([C, GW], F32, tag="pB", name="ps")
            for cl in range(GCH):
                i = g * GCH + cl
                nc.tensor.matmul(ps[:, ts(cl, C)], kT_all[:, ts(i, C)],
                                 qT_all[:, ts(i, C)], start=True, stop=True)
            sT_b = spool.tile([C, GW], BF16, tag=f"sm{g}", name="sT_b")
            nc.vector.tensor_mul(sT_b[:], ps[:], mlocT4[:])
            qTeo = spool.tile([2 * K, GW], BF16, tag=f"qeo{g}", name="qTeo")
            nc.vector.tensor_mul(qTeo[0:K, :], qT_all[:, ts(g, GW)], cemat[:])
            nc.gpsimd.tensor_mul(qTeo[K : 2 * K, :], qT_all[:, ts(g, GW)], comat[:])
            return sT_b, qTeo

        def attn_out_chunk(g, cl, v_b, sT_b, qTeo):
            i = g * GCH + cl
            plt = pbig.tile([V, C], F32, tag="pB", name=f"plt{i}")
            nc.tensor.matmul(plt[:], v_b[:, ts(cl, V)], sT_b[:, ts(cl, C)],
                             start=True, stop=False)
            nc.tensor.matmul(plt[:], geo_bf[:, ts(i, V)], qTeo[:, ts(cl, C)],
                             start=False, stop=True)
            if cl % 2 == 0:
                nc.scalar.copy(lt_all[:, ts(i, C)], plt[:])
            else:
                nc.vector.tensor_copy(lt_all[:, ts(i, C)], plt[:])
            out_sb = opool.tile([C, E], BF16, tag="osb", name=f"out_sb{i}")
            for h in range(2):
                po = pbig.tile([C, GW], F32, tag="pB", name=f"po{i}_{h}")
                nc.tensor.matmul(po[:], lt_all[:, ts(i, C)],
                                 wo[:, ts(h, GW)], start=True, stop=True)
                dst = out_sb[:, ts(h, GW)]
                if h == 0:
                    nc.scalar.copy(dst, po[:])
                else:
                    nc.vector.tensor_copy(dst, po[:])
            nc.sync.dma_start(out_d[i], out_sb[:])

        # ============ g0 projections, then prefix state, then g1 ==========
        proj_group(0)

        kn1, v1_b = transposes(kT1, vT1, 1, "1")
        kgeo1 = kgeo_stage(kn1, 1, "1")
        pu1 = pproj.tile([2 * K, V], F32, tag="pP", name="pu1")
        nc.tensor.matmul(pu1[:], kgeo1[:], v1_b[:], start=True, stop=True)
        nc.vector.tensor_scalar_mul(geo_all[:, 0:V], pu1[:],
                                    gamma[:, G_GAMMA : G_GAMMA + 1])
        nc.vector.tensor_copy(geo_bf[:, 0:V], geo_all[:, 0:V])

        proj_group(1)

        # ============ per-group: transposes, scores, state, attn, out ======
        pu2 = pstate.tile([2 * K, (NCH - 1) * V], F32, name="pu2")

        def chain_step(i):
            nc.vector.scalar_tensor_tensor(
                geo_all[:, ts(i, V)], geo_all[:, ts(i - 1, V)], DC2,
                pu2[:, ts(i - 1, V)], AluOpType.mult, AluOpType.add,
            )
            nc.vector.tensor_copy(geo_bf[:, ts(i, V)], geo_all[:, ts(i, V)])

        for g in range(NG):
            kn_g, v_g = transposes(kT_all[:, ts(g, GW)], vT_sbs[g], GCH, f"2{g}")
            sT_b, qTeo = scores_stage(g)
            kgeo_g = kgeo_stage(kn_g, GCH, f"2{g}")
            for cl in range(GCH):
                i = g * GCH + cl
                if i < NCH - 1:
                    nc.tensor.matmul(pu2[:, ts(i, V)], kgeo_g[:, ts(cl, 2 * K)],
                                     v_g[:, ts(cl, V)], start=True, stop=True)
            for cl in range(GCH):
                i = g * GCH + cl
                if i > 0:
                    chain_step(i)
                attn_out_chunk(g, cl, v_g, sT_b, qTeo)

    nc.compile()
    return nc


_CACHE = {}


def _get_program(has_bv):
    key = ("nc", has_bv)
    if key not in _CACHE:
        _CACHE[key] = _build_program(has_bv)
    return _CACHE[key]


def _make_in_maps(x, Wk, bk, Wv, bv, Wq, bq, Wo):
    import ml_dtypes

    bfd = ml_dtypes.bfloat16
    consts = _host_constants()

    def pack2(Wa, Wb):
        # [128, NEC*(outA+outB)]: per embed sub-chunk, [Wa_ec | Wb_ec] columns
        Wab = np.concatenate(
            [Wa.T.reshape(NEC, 128, -1), Wb.T.reshape(NEC, 128, -1)], 2
        )
        return np.ascontiguousarray(
            Wab.transpose(1, 0, 2).reshape(128, -1)
        ).astype(bfd)

    def pack1(W):
        return np.ascontiguousarray(
            W.T.reshape(NEC, 128, -1).transpose(1, 0, 2).reshape(128, -1)
        ).astype(bfd)

    identhi = np.zeros((128, 64), np.float32)
    identhi[64:128, :] = np.eye(64)
    cid = np.zeros((128, IDB_W), np.float32)
    chot = np.zeros((128, HOT_W), np.float32)

    def setreg(pack, layout, name, arr):
        r, o, c = layout[name]
        pack[0:r, o : o + c] = arr

    setreg(cid, _IDB, "ident", consts["ident64"])
    setreg(cid, _IDB, "identhi", identhi)
    setreg(chot, _HOT, "mlocT4", consts["mlocT4"])
    setreg(chot, _HOT, "cemat", consts["cemat"])
    setreg(chot, _HOT, "comat", consts["comat"])

    shared = {
        "cid": cid.astype(bfd),
        "chot": chot.astype(bfd),
        "woT": np.ascontiguousarray(Wo.T).astype(bfd),
        "wpx": np.concatenate([pack2(Wk, Wv), pack1(Wq)], 1),
    }

    def pack_x(xh):
        # [E, HT] -> [NG*128, (ec, half, t)]: one contiguous block per group
        HW2 = GW // 2
        v = xh.reshape(NEC, 128, NG, 2, HW2).transpose(2, 1, 0, 3, 4)
        return np.ascontiguousarray(v.reshape(NG * 128, NEC * GW)).astype(bfd)

    def pack_pre(xh):
        # last PRE prefix rows -> [128, (ec, t)] contiguous block
        v = xh[:, HT - PRE :].reshape(NEC, 128, PRE).transpose(1, 0, 2)
        return np.ascontiguousarray(v.reshape(128, NEC * PRE)).astype(bfd)

    gcol = np.zeros((128, G_NCOL), np.float32)
    gcol[0:K, G_BKV] = bk
    gcol[K : 2 * K, G_BKV] = bv
    gcol[0:K, G_BQ] = bq
    gcol[:, G_WGE] = consts["wge"]
    gcol[:, G_WGO] = consts["wgo"]

    zeros_pre = np.zeros((128, NEC * PRE), bfd)
    in_maps = []
    for c in range(N_CORES):
        b, h = c // 2, c % 2
        xbT = np.ascontiguousarray(x[b].T)  # [E, T]
        m = dict(shared)
        m["xpre"] = pack_pre(xbT[:, :HT]) if h == 1 else zeros_pre
        m["xq2"] = pack_x(xbT[:, h * HT : (h + 1) * HT])
        g = gcol.copy()
        g[:, G_GAMMA] = float(h)
        m["gamma_col"] = g
        in_maps.append(m)
    return in_maps


def run(inputs, trace=False):
    """Run on 8 cores; returns (output, BassKernelResults)."""
    inp = {k: np.asarray(v) for k, v in inputs.items()}
    has_bv = bool(np.any(inp["bv"]))
    nc = _get_program(has_bv)
    in_maps = _make_in_maps(**inp)
    res = bass_utils.run_bass_kernel_spmd(
        nc, in_maps, core_ids=list(range(N_CORES)), trace=trace
    )
    out = np.empty((B, T, E), np.float32)
    for c in range(N_CORES):
        b, h = c // 2, c % 2
        o = res.results[c]["out"].astype(np.float32)  # [NCH, 128, E]
        out[b, h * HT : (h + 1) * HT, :] = o.reshape(HT, E)
    return out, res


def kernel(**inputs):
    out, _ = run(inputs, trace=False)
    return out


# revision 8
# speedup vs baseline: 1.0721x; 1.0721x over previous
"""Trainium2 Bass kernel for nn_Decay2D (decay-masked linear attention).

Math: the reference's Hillis-Steele scan with decay-squaring order composes
to coefficient d^ceil((t-s)/2) on store[s] = scale*k_s v_s^T, so

    out[t] = scale^2 * sum_{s<=t} d^ceil((t-s)/2) (q_t . k_s) v_s  @ Wo^T

computed as chunked linear attention with a [K, 2V] carry state per chunk
(even/odd decay chains on the V axis), never materializing [B,T,K,V].

Sharding: 8 cores = 4 batches x 2 sequence halves. Each core builds the
carry state over a truncated 128-row prefix and runs full attention +
output projection for its own 1024 rows.

v5: the body is built around the DMA stream. All transfers are contiguous
DRAM blocks (one header with weights/prefix/idents, per-(group,half) x
blocks, one consts+Wo block, per-chunk output stores) on the sync HWDGE
ring in consumption order; the PE chases the stream. The carry state keeps
parity on the V axis so every elementwise op is contiguous: ve/vo = v *
wge/wgo (per-partition scalars), pu2 = kn^T @ [ve|vo], and attention adds
geo_e^T qTe + geo_o^T qTo.
"""

from contextlib import ExitStack

import numpy as np

import concourse.bass as bass
import concourse.bacc as bacc
import concourse.mybir as mybir
import concourse.tile as tile
from concourse import bass_utils
from concourse.alu_op_type import AluOpType
from concourse.bass import ts

F32 = mybir.dt.float32
BF16 = mybir.dt.bfloat16
SIG = mybir.ActivationFunctionType.Sigmoid

B, T, E, K, V = 4, 2048, 1024, 64, 64
DECAY = 0.9
C = 128          # chunk length
HT = T // 2      # rows per core (sequence half)
NCH = HT // C    # chunks per half (8)
NEC = E // 128   # embed sub-chunks (8)
GW = 512         # group width: 4 chunks per PSUM bank
GCH = GW // C    # chunks per group (4)
NG = HT // GW    # groups per half (2)
HW2 = GW // 2    # half-group width (256)
HCOL = NEC * HW2 # x columns per (group, half) block (2048)
DC2 = float(DECAY ** (C // 2))
N_CORES = 8
PRE = 128        # truncated prefix length (1 chunk; older rows decay < 2e-3)

# header block: packed Wk|Wv, Wq, prefix x, ident blocks
def _mklayout(regions):
    out, off = {}, 0
    for n, r, c in regions:
        out[n] = (r, off, c)
        off += c
    return out, off


_HDR, HDR_W = _mklayout([
    ("wkv", 128, NEC * 2 * K), ("wq", 128, NEC * K),
    ("xpre", 128, NEC * PRE), ("ident", 64, 64), ("identhi", 128, 64),
])
_CW, CW_W = _mklayout([
    ("mlocT4", C, GW), ("cemat", K, GW), ("comat", K, GW), ("wo", K, E),
])

# gamma columns: 0 prefix flag, 1 bk|bv, 2 bq, 3 wge, 4 wgo
G_GAMMA, G_BKV, G_BQ, G_WGE, G_WGO = 0, 1, 2, 3, 4
G_NCOL = 5


def _host_constants():
    d = DECAY
    scale2 = 1.0 - d
    i = np.arange(C)
    j = np.arange(C)
    delta = i[:, None] - j[None, :]
    # intra-chunk decay mask, transposed to [tcol(j), trow(i)], scale^2 folded
    mloc = np.where(delta >= 0, d ** np.ceil(delta / 2.0), 0.0) * scale2
    mlocT4 = np.tile(np.ascontiguousarray(mloc.T), (1, GCH)).astype(np.float32)
    # boundary coefficient per local row i (scale^2 folded), split by parity
    c = d ** np.ceil((i + 1) / 2.0) * scale2
    ce = np.where(i % 2 == 0, c, 0.0).astype(np.float32)
    co = np.where(i % 2 == 1, c, 0.0).astype(np.float32)
    cemat = np.tile(np.broadcast_to(ce, (K, C)), (1, GCH)).astype(np.float32)
    comat = np.tile(np.broadcast_to(co, (K, C)), (1, GCH)).astype(np.float32)
    # state-update row weights (per t within chunk)
    u_o = np.where(j % 2 == 1, d ** ((C - 1 - j) / 2.0), 0.0)
    u_e = np.where(j % 2 == 0, d ** ((C - 2 - j) / 2.0), 0.0)
    wge = (u_o + u_e).astype(np.float32)          # [C]
    wgo = (u_o + d * u_e).astype(np.float32)
    return {
        "mlocT4": mlocT4,
        "cemat": np.ascontiguousarray(cemat),
        "comat": np.ascontiguousarray(comat),
        "wge": wge,
        "wgo": wgo,
        "ident64": np.eye(64, dtype=np.float32),
    }


def _build_program(has_bv):
    nc = bacc.Bacc(
        "TRN2",
        debug=False,
        enable_asserts=False,
        target_bir_lowering=False,
        num_devices=N_CORES,
    )

    def din(name, shape, dtype=F32):
        return nc.dram_tensor(name, shape, dtype, kind="ExternalInput").ap()

    gamma_d = din("gamma_col", [128, G_NCOL])
    hdr_d = din("hdr", [128, HDR_W], BF16)
    xq4 = din("xq4", [NG * 2, 128, HCOL], BF16)   # x per (group, half), (ec, t)
    cw_d = din("chotwo", [128, CW_W], BF16)       # mloc / ce / co / Wo^T
    out_d = nc.dram_tensor("out", [NCH, 128, E], BF16,
                           kind="ExternalOutput").ap()

    with ExitStack() as ctx:
        tc = ctx.enter_context(tile.TileContext(nc))

        consts = ctx.enter_context(tc.tile_pool(name="consts", bufs=1))
        state = ctx.enter_context(tc.tile_pool(name="state", bufs=1))
        xpool = ctx.enter_context(tc.tile_pool(name="xg", bufs=2))
        spool = ctx.enter_context(tc.tile_pool(name="sml", bufs=2))
        opool = ctx.enter_context(tc.tile_pool(name="osb", bufs=3))
        # PSUM budget (8 banks): pproj ring 2, pbig ring 2, psml ring 2,
        # pstate 2 (pu2 [K, 7*2V] f32 spans 2 banks, lives across groups)
        pproj = ctx.enter_context(tc.tile_pool(name="pproj", bufs=2, space="PSUM"))
        pbig = ctx.enter_context(tc.tile_pool(name="pbig", bufs=2, space="PSUM"))
        psml = ctx.enter_context(tc.tile_pool(name="psml", bufs=2, space="PSUM"))
        pstate = ctx.enter_context(tc.tile_pool(name="pstate", bufs=1, space="PSUM"))

        # ---- loads in wire order (single sync HWDGE ring = FIFO) ----
        gamma = consts.tile([128, G_NCOL], F32, name="gamma_sb")
        nc.sync.dma_start(gamma[:], gamma_d[:])
        hdr = consts.tile([128, HDR_W], BF16, name="hdr")
        nc.sync.dma_start(hdr[:], hdr_d[:])

        xg2s = [xpool.tile([128, 2 * HCOL], BF16, tag="xg", name=f"xg2_{g}")
                for g in range(NG)]

        def load_xhalf(g, hf):
            nc.sync.dma_start(
                xg2s[g][:, hf * HCOL : (hf + 1) * HCOL], xq4[g * 2 + hf])

        load_xhalf(0, 0)
        load_xhalf(0, 1)
        cw = consts.tile([128, CW_W], BF16, name="cw")
        nc.sync.dma_start(cw[:], cw_d[:])
        load_xhalf(1, 0)
        load_xhalf(1, 1)

        def reg(pack, layout, name):
            r, o, c = layout[name]
            return pack[0:r, o : o + c]

        wkv, wq = reg(hdr, _HDR, "wkv"), reg(hdr, _HDR, "wq")
        xp = reg(hdr, _HDR, "xpre")
        ident, identhi = reg(hdr, _HDR, "ident"), reg(hdr, _HDR, "identhi")
        mlocT4 = reg(cw, _CW, "mlocT4")
        cemat, comat = reg(cw, _CW, "cemat"), reg(cw, _CW, "comat")
        wo = reg(cw, _CW, "wo")
        bk_ap = gamma[0:K, G_BKV : G_BKV + 1]
        bv_ap = gamma[K : 2 * K, G_BKV : G_BKV + 1]
        bq_ap = gamma[0:K, G_BQ : G_BQ + 1]
        wge_ap = gamma[:, G_WGE : G_WGE + 1]
        wgo_ap = gamma[:, G_WGO : G_WGO + 1]

        qT_all = consts.tile([K, HT], BF16, name="qT_all")
        kT_all = consts.tile([K, HT], BF16, name="kT_all")
        lt_all = consts.tile([V, HT], BF16, name="lt_all")
        # carry state: parity on the V axis -> [K, NCH * 2V]
        geo_all = state.tile([K, NCH * 2 * V], F32, name="geo_all")
        geo_bf = state.tile([K, NCH * 2 * V], BF16, name="geo_bf")

        def ts2(i):  # [K, 2V] slice of the state for chunk i
            return slice(i * 2 * V, (i + 1) * 2 * V)

        # ============ prefix projection (1 chunk) ============
        pkv1 = pproj.tile([2 * K, PRE], F32, tag="pP", name="pkv1")
        for ec in range(NEC):
            nc.tensor.matmul(pkv1[:], wkv[:, ts(ec, 2 * K)], xp[:, ts(ec, PRE)],
                             start=(ec == 0), stop=(ec == NEC - 1))
        kT1 = spool.tile([K, PRE], BF16, tag="kT1", name="kT1")
        nc.scalar.activation(kT1[:], pkv1[0:K, :], SIG, bias=bk_ap)
        vT1 = spool.tile([2 * K, PRE], BF16, tag="vT1", name="vT1")
        nc.scalar.copy(vT1[K : 2 * K, :], pkv1[K : 2 * K, :])
        if has_bv:
            nc.vector.tensor_scalar_add(vT1[K : 2 * K, :], vT1[K : 2 * K, :], bv_ap)

        # ============ group projections (PE chases the x stream) ============
        vT_sbs = []

        def proj_group(g):
            pkv = pproj.tile([2 * K, GW], F32, tag="pP", name=f"pkv_{g}")
            pqg = pproj.tile([K, GW], F32, tag="pP", name=f"pq_{g}")
            for hf in range(2):
                xh = xg2s[g][:, hf * HCOL : (hf + 1) * HCOL]
                dst = slice(hf * HW2, hf * HW2 + HW2)
                for ec in range(NEC):
                    nc.tensor.matmul(pkv[0 : 2 * K, dst], wkv[:, ts(ec, 2 * K)],
                                     xh[:, ts(ec, HW2)],
                                     start=(ec == 0), stop=(ec == NEC - 1))
                for ec in range(NEC):
                    nc.tensor.matmul(pqg[0:K, dst], wq[:, ts(ec, K)],
                                     xh[:, ts(ec, HW2)],
                                     start=(ec == 0), stop=(ec == NEC - 1))
            nc.scalar.activation(kT_all[:, ts(g, GW)], pkv[0:K, :], SIG, bias=bk_ap)
            vT_sb = spool.tile([2 * K, GW], BF16, tag=f"vT{g}", name=f"vT_sb{g}")
            nc.scalar.copy(vT_sb[K : 2 * K, :], pkv[K : 2 * K, :])
            if has_bv:
                nc.vector.tensor_scalar_add(
                    vT_sb[K : 2 * K, :], vT_sb[K : 2 * K, :], bv_ap)
            vT_sbs.append(vT_sb)
            nc.scalar.activation(qT_all[:, ts(g, GW)], pqg[:], SIG, bias=bq_ap)

        # ---- stage helpers ----
        def transposes(kT_src, vT_src, nch, tagp):
            pkn = psml.tile([C, nch * K], BF16, tag="pS", name=f"pkn{tagp}")
            for cl in range(nch):
                nc.tensor.matmul(pkn[:, ts(cl, K)], kT_src[:, ts(cl, C)],
                                 ident[:], is_transpose=True)
            pvn = psml.tile([C, nch * V], BF16, tag="pS", name=f"pvn{tagp}")
            for cl in range(nch):
                nc.tensor.matmul(pvn[:, ts(cl, V)], vT_src[K : 2 * K, ts(cl, C)],
                                 identhi[K : 2 * K, :], is_transpose=True)
            kn = spool.tile([C, nch * K], BF16, tag=f"kn{tagp}", name=f"kn{tagp}")
            nc.scalar.copy(kn[:], pkn[:])
            v_b = spool.tile([C, nch * V], BF16, tag=f"v{tagp}", name=f"v{tagp}")
            nc.vector.tensor_copy(v_b[:], pvn[:])
            # parity-weighted v for the state update
            veo = spool.tile([C, 2 * nch * V], BF16, tag=f"veo{tagp}",
                             name=f"veo{tagp}")
            nc.vector.tensor_scalar_mul(veo[:, 0 : nch * V], v_b[:], wge_ap)
            nc.vector.tensor_scalar_mul(veo[:, nch * V :], v_b[:], wgo_ap)
            return kn, v_b, veo

        def scores_stage(g):
            ps = pbig.tile([C, GW], F32, tag="pB", name="ps")
            for cl in range(GCH):
                i = g * GCH + cl
                nc.tensor.matmul(ps[:, ts(cl, C)], kT_all[:, ts(i, C)],
                                 qT_all[:, ts(i, C)], start=True, stop=True)
            sT_b = spool.tile([C, GW], BF16, tag=f"sm{g}", name="sT_b")
            nc.vector.tensor_mul(sT_b[:], ps[:], mlocT4[:])
            qTe = spool.tile([K, GW], BF16, tag=f"qe{g}", name="qTe")
            nc.vector.tensor_mul(qTe[:], qT_all[:, ts(g, GW)], cemat[:])
            qTo = spool.tile([K, GW], BF16, tag=f"qo{g}", name="qTo")
            nc.gpsimd.tensor_mul(qTo[:], qT_all[:, ts(g, GW)], comat[:])
            return sT_b, qTe, qTo

        def attn_out_chunk(g, cl, v_b, sT_b, qTe, qTo):
            i = g * GCH + cl
            plt = pbig.tile([V, C], F32, tag="pB", name=f"plt{i}")
            nc.tensor.matmul(plt[:], v_b[:, ts(cl, V)], sT_b[:, ts(cl, C)],
                             start=True, stop=False)
            nc.tensor.matmul(plt[:], geo_bf[:, i * 2 * V : i * 2 * V + V],
                             qTe[:, ts(cl, C)], start=False, stop=False)
            nc.tensor.matmul(plt[:], geo_bf[:, i * 2 * V + V : (i + 1) * 2 * V],
                             qTo[:, ts(cl, C)], start=False, stop=True)
            if cl % 2 == 0:
                nc.scalar.copy(lt_all[:, ts(i, C)], plt[:])
            else:
                nc.vector.tensor_copy(lt_all[:, ts(i, C)], plt[:])
            out_sb = opool.tile([C, E], BF16, tag="osb", name=f"out_sb{i}")
            for h in range(2):
                po = pbig.tile([C, GW], F32, tag="pB", name=f"po{i}_{h}")
                nc.tensor.matmul(po[:], lt_all[:, ts(i, C)],
                                 wo[:, ts(h, GW)], start=True, stop=True)
                dst = out_sb[:, ts(h, GW)]
                if h == 0:
                    nc.scalar.copy(dst, po[:])
                else:
                    nc.vector.tensor_copy(dst, po[:])
            nc.sync.dma_start(out_d[i], out_sb[:])

        # ============ g0 projections, then prefix state, then g1 ==========
        proj_group(0)

        kn1, v1_b, veo1 = transposes(kT1, vT1, 1, "1")
        pu1 = pproj.tile([K, 2 * V], F32, tag="pP", name="pu1")
        nc.tensor.matmul(pu1[:, 0:V], kn1[:], veo1[:, 0:V], start=True, stop=True)
        nc.tensor.matmul(pu1[:, V : 2 * V], kn1[:], veo1[:, V : 2 * V],
                         start=True, stop=True)
        nc.vector.tensor_scalar_mul(geo_all[:, ts2(0)], pu1[:],
                                    gamma[0:K, G_GAMMA : G_GAMMA + 1])
        nc.vector.tensor_copy(geo_bf[:, ts2(0)], geo_all[:, ts2(0)])

        proj_group(1)

        # ============ per-group: transposes, scores, state, attn, out ======
        pu2 = pstate.tile([K, (NCH - 1) * 2 * V], F32, name="pu2")

        def chain_step(i):
            nc.vector.scalar_tensor_tensor(
                geo_all[:, ts2(i)], geo_all[:, ts2(i - 1)], DC2,
                pu2[:, ts2(i - 1)], AluOpType.mult, AluOpType.add,
            )
            nc.vector.tensor_copy(geo_bf[:, ts2(i)], geo_all[:, ts2(i)])

        for g in range(NG):
            kn_g, v_g, veo_g = transposes(kT_all[:, ts(g, GW)], vT_sbs[g],
                                          GCH, f"2{g}")
            sT_b, qTe, qTo = scores_stage(g)
            for cl in range(GCH):
                i = g * GCH + cl
                if i < NCH - 1:
                    nc.tensor.matmul(pu2[:, i * 2 * V : i * 2 * V + V],
                                     kn_g[:, ts(cl, K)],
                                     veo_g[:, ts(cl, V)],
                                     start=True, stop=True)
                    nc.tensor.matmul(pu2[:, i * 2 * V + V : (i + 1) * 2 * V],
                                     kn_g[:, ts(cl, K)],
                                     veo_g[:, (GCH + cl) * V : (GCH + cl + 1) * V],
                                     start=True, stop=True)
            for cl in range(GCH):
                i = g * GCH + cl
                if i > 0:
                    chain_step(i)
                attn_out_chunk(g, cl, v_g, sT_b, qTe, qTo)

    nc.compile()
    return nc


_CACHE = {}


def _get_program(has_bv):
    key = ("nc", has_bv)
    if key not in _CACHE:
        _CACHE[key] = _build_program(has_bv)
    return _CACHE[key]


def _make_in_maps(x, Wk, bk, Wv, bv, Wq, bq, Wo):
    import ml_dtypes

    bfd = ml_dtypes.bfloat16
    consts = _host_constants()

    def pack2(Wa, Wb):
        # [128, NEC*(outA+outB)]: per embed sub-chunk, [Wa_ec | Wb_ec] columns
        Wab = np.concatenate(
            [Wa.T.reshape(NEC, 128, -1), Wb.T.reshape(NEC, 128, -1)], 2
        )
        return np.ascontiguousarray(
            Wab.transpose(1, 0, 2).reshape(128, -1)
        )

    def pack1(W):
        return np.ascontiguousarray(
            W.T.reshape(NEC, 128, -1).transpose(1, 0, 2).reshape(128, -1)
        )

    identhi = np.zeros((128, 64), np.float32)
    identhi[64:128, :] = np.eye(64)

    def setreg(pack, layout, name, arr):
        r, o, c = layout[name]
        pack[0:r, o : o + c] = arr

    cw = np.zeros((128, CW_W), np.float32)
    setreg(cw, _CW, "mlocT4", consts["mlocT4"])
    setreg(cw, _CW, "cemat", consts["cemat"])
    setreg(cw, _CW, "comat", consts["comat"])
    setreg(cw, _CW, "wo", Wo.T)

    hdr = np.zeros((128, HDR_W), np.float32)
    setreg(hdr, _HDR, "wkv", pack2(Wk, Wv))
    setreg(hdr, _HDR, "wq", pack1(Wq))
    setreg(hdr, _HDR, "ident", consts["ident64"])
    setreg(hdr, _HDR, "identhi", identhi)

    shared = {"chotwo": cw.astype(bfd)}

    def pack_x(xh):
        # [E, HT] -> [NG*2, 128, (ec, t)]: one contiguous block per (g, half)
        v = xh.reshape(NEC, 128, NG, 2, HW2).transpose(2, 3, 1, 0, 4)
        return np.ascontiguousarray(v.reshape(NG * 2, 128, HCOL)).astype(bfd)

    def pack_pre(xh):
        # last PRE prefix rows -> [128, (ec, t)] contiguous block
        v = xh[:, HT - PRE :].reshape(NEC, 128, PRE).transpose(1, 0, 2)
        return np.ascontiguousarray(v.reshape(128, NEC * PRE))

    gcol = np.zeros((128, G_NCOL), np.float32)
    gcol[0:K, G_BKV] = bk
    gcol[K : 2 * K, G_BKV] = bv
    gcol[0:K, G_BQ] = bq
    gcol[:, G_WGE] = consts["wge"]
    gcol[:, G_WGO] = consts["wgo"]

    in_maps = []
    for c in range(N_CORES):
        b, h = c // 2, c % 2
        xbT = np.ascontiguousarray(x[b].T)  # [E, T]
        m = dict(shared)
        hh = hdr.copy()
        if h == 1:
            setreg(hh, _HDR, "xpre", pack_pre(xbT[:, :HT]))
        m["hdr"] = hh.astype(bfd)
        m["xq4"] = pack_x(xbT[:, h * HT : (h + 1) * HT])
        g = gcol.copy()
        g[:, G_GAMMA] = float(h)
        m["gamma_col"] = g
        in_maps.append(m)
    return in_maps


def run(inputs, trace=False):
    """Run on 8 cores; returns (output, BassKernelResults)."""
    inp = {k: np.asarray(v) for k, v in inputs.items()}
    has_bv = bool(np.any(inp["bv"]))
    nc = _get_program(has_bv)
    in_maps = _make_in_maps(**inp)
    res = bass_utils.run_bass_kernel_spmd(
        nc, in_maps, core_ids=list(range(N_CORES)), trace=trace
    )
    out = np.empty((B, T, E), np.float32)
    for c in range(N_CORES):
        b, h = c // 2, c % 2
        o = res.results[c]["out"].astype(np.float32)  # [NCH, 128, E]
        out[b, h * HT : (h + 1) * HT, :] = o.reshape(HT, E)
    return out, res


def kernel(**inputs):
    out, _ = run(inputs, trace=False)
    return out
